# revision 39
# baseline (speedup 1.0000x reference)
"""Multi-head attention (B=4, N=2048, DIM=768, H=12) on 8 TRN2 cores.

Sharding: core c -> batch c//2, heads (c%2)*6 .. +6  (6 heads = 3 pairs).
Each core computes its heads' attention and a partial output projection
(row-sharded w_proj); host sums the two partials per batch and adds bias.

Per-core dataflow (bf16 matmuls, fp16 softmax weights/values):
  inputs : xt [768,2048] (= x[b].T) column-block-first on the sync ring;
           wq/wk/wv/wp host-premerged to [128, chunks*K], pair-0 columns
           first (strided DMA) on the scalar ring so the first qkv group
           unblocks early
  qkv    : Q^T,K^T per head-pair [128,2048] (d-major), V token-major per
           pair as [V_even | 1s(64) | V_odd] (192 cols, shared ones-block)
  scores : S^T[keys, q], 2 heads row-tiled per key-tile matmul (the pair
           streams concurrently in the 64x128 array tiles)
  softmax: exp on ACT, fp16 out (scale folded in; max-subtraction skipped -
           scores O(5))
  PV     : head-a lhsT = [V|1s] -> PSUM [P1 ; den1 x64], head-b lhsT =
           [1s|V] -> [den2 x64 ; P2]: denominators come out pre-replicated
           across 64 partitions, so normalization is one full-tile fast
           reciprocal (custom-DVE, base partition 0 only), two small
           SBUF->SBUF DMA partition shifts of the reciprocal rows, and two
           lane-aligned multiplies writing ot directly (no gpsimd
           broadcast, no row hops, no head-2 output shift)
  proj   : partial = OT.T-slices @ wp, PSUM halves on the dr tag

Schedule: the stage-1A Tile scheduler replays engines in emission order,
popping ready work greedily.  kt steps are processed in groups of 2 (4
score MMs then 4 PV MMs) and the attention core is SOFTWARE-PIPELINED one
group deep: each PV group is emitted after the NEXT scores group, so a new
block's first scores precede the previous block's last PVs and the ACT exp
stream (the co-wall with the PE) never stalls at block boundaries.  Bulk
PE filler (later pairs' qkv, proj groups, the previous block's norm chain)
is woven between groups.  U banks are evacuated with one full-tile fp32
copy each (DVE cost is free-size, so [128,512] costs the same as
[64,512]); the final block norms straight out of PSUM to shorten the tail
latency chain.
"""

import sys

for _p in ("/opt/trn_rl_repo",):
    if _p not in sys.path:
        sys.path.insert(0, _p)

import numpy as np
import ml_dtypes

import concourse.bass as bass
import concourse.bacc as bacc
import concourse.mybir as mybir
import concourse.tile as tile
from concourse.bass_utils import run_bass_kernel_spmd

# ---- custom-DVE exp: exp(s/8) = (e^(s/512))^64 ------------------------------
# Two vector-engine ops so part of the softmax exp stream can run on the DVE
# (the ACT engine is 100% busy mid-kernel, the DVE ~35%).  EXP_P3_ANT is the
# cubic Taylor of e^(s/512) (error^64 < 1e-3 at 8 sigma of the score
# distribution, ~4e-5 at 4 sigma); SQ6_ANT raises to the 64th power by
# squaring.  Registered at import into the per-NEFF DVE table.
import concourse.dve_ops as _do
from concourse.dve_spec import Spec as _Spec, Src0 as _S0, C0 as _C0, \
    C1 as _C1, C2 as _C2, One as _One, sq as _sq


def _register_dve(op):
    if op.name not in _do._SUB_OPCODE_FOR_NAME:
        _do.OPS.append(op)
        _do.CUSTOM_DVE_SPECS[op.name] = op.spec
        _do._SUB_OPCODE_FOR_NAME[op.name] = (
            _do._CUSTOM_DVE_ROW_BASE + len(_do.OPS) - 1)
        assert _do._SUB_OPCODE_FOR_NAME[op.name] < 0x20
    return op


EXP_P3_ANT = _register_dve(_do.DveOp(
    "EXP_P3_ANT",
    _Spec(body=((_S0 * _C0 + _C1) * _S0 + _C2) * _S0 + _One,
          reference=lambda in0, s0, s1, imm2:
          ((in0 * s0 + s1) * in0 + imm2) * in0 + 1.0),
    subdim=False,
    uops_sha={"v3": "64e152996f8449e5", "v4": "a77d1908c14d14ab"},
))
SQ6_ANT = _register_dve(_do.DveOp(
    "SQ6_ANT",
    _Spec(body=_sq(_sq(_sq(_sq(_sq(_sq(_S0)))))),
          reference=lambda in0, s0, s1, imm2: in0 ** 64),
    subdim=False,
    uops_sha={"v3": "8add6fae2d93d0d2", "v4": "acddb5876245b3f8"},
))
_EXP_R = 512.0
_EXP_A1 = 1.0 / _EXP_R
_EXP_A2 = 1.0 / (2.0 * _EXP_R * _EXP_R)
_EXP_A3 = 1.0 / (6.0 * _EXP_R ** 3)
VEC_EXP_COLS = 128   # exp columns per step computed on the DVE instead of ACT

DIM = 768
HEADS = 12
HD = 64
B = 4
N = 2048
NCORES = 8
PAIRS = 3          # head-pairs per core (6 heads)
CH = DIM // 128    # 6 contraction chunks of 128
KT = N // 128      # 16 key tiles
QB = N // 512      # 4 query blocks of 512
F32 = mybir.dt.float32
EXP = mybir.ActivationFunctionType.Exp
SCALE = HD ** -0.5

DEFAULT_DTYPE = "bf16"


def build_program(dtype="f32", debug=False, overlap=None):
    if overlap is None:
        overlap = dtype != "f32"
    dt = F32 if dtype == "f32" else mybir.dt.bfloat16
    nc = bacc.Bacc()
    # weights arrive host-premerged: w*[r, c*384+k] = W[c*128+r, k] so each
    # is ONE contiguous [128, 2304] DMA
    xt = nc.declare_dram_parameter("xt", [DIM, N], dt, isOutput=False)
    wq = nc.declare_dram_parameter("wq", [128, CH * PAIRS * 128], dt, isOutput=False)
    wk = nc.declare_dram_parameter("wk", [128, CH * PAIRS * 128], dt, isOutput=False)
    wv = nc.declare_dram_parameter("wv", [128, CH * PAIRS * 128], dt, isOutput=False)
    wp = nc.declare_dram_parameter("wp", [128, PAIRS * DIM], dt, isOutput=False)
    out = nc.declare_dram_parameter("out", [N, DIM], dt, isOutput=True)
    dbg = None
    if debug:
        dbg = {
            "dbg_v0": nc.declare_dram_parameter("dbg_v0", [128, 390], F32, isOutput=True),
            "dbg_den": nc.declare_dram_parameter("dbg_den", [1, 1024], F32, isOutput=True),
            "dbg_dsb": nc.declare_dram_parameter("dbg_dsb", [1, 1024], F32, isOutput=True),
            "dbg_e0": nc.declare_dram_parameter("dbg_e0", [128, 1024], F32, isOutput=True),
        }

    with tile.TileContext(nc) as tc:
        emit(tc, nc, xt, wq, wk, wv, wp, out, dt, overlap=overlap, dbg=dbg)
    nc.compile()
    return nc


def emit(tc, nc, xt, wq, wk, wv, wp, out, dt, overlap, dbg=None):
    import contextlib

    ctx = contextlib.ExitStack()
    wbufs = 2 if overlap else 1
    with ctx:
        sb = ctx.enter_context(tc.tile_pool(name="sb", bufs=1))
        ps = ctx.enter_context(tc.tile_pool(name="ps", bufs=1, space="PSUM"))

        if overlap:
            # PE warm-up: ~3.4us of back-to-back dummy matmuls while the
            # input DMAs stream, so the HAM clock gate is already at 8/8
            # (2.4 GHz) when the first real matmul issues (cold PE runs at
            # 1.2 GHz for the first ~3.4us of activity otherwise). Emitted
            # FIRST so its memset precedes all other DVE work.
            warm = sb.tile([128, 8], dt, name="warm", tag="warm")
            nc.vector.memset(warm[:], 0.0)
            wps = ps.tile([128, 512], F32, name="dr", tag="dr", bufs=2)
            for i in range(70):
                nc.tensor.matmul(wps[0:8, 0:8], lhsT=warm[:, 0:8],
                                 rhs=warm[:, 0:8], start=True, stop=True)

        # ---- load inputs -------------------------------------------------
        # Weight tensors in dependency order (wq -> wk -> wv -> wp), each ONE
        # contiguous DMA on the scalar ring; xt column-block-first on the
        # sync ring so the first qkv group (needs cols 0:512 of all 6
        # chunks) unblocks at ~2us instead of ~12us.
        def load_w(dram, nm, p0_first=False):
            t = sb.tile([128, CH * PAIRS * 128], dt, name=nm, tag=nm)
            if p0_first:
                # pair-0's chunk columns first (one strided DMA) so the very
                # first qkv group unblocks before the bulk arrives
                d3 = dram.rearrange("p (c k) -> p c k", k=PAIRS * 128)
                t3 = t.rearrange("p (c k) -> p c k", k=PAIRS * 128)
                nc.scalar.dma_start(out=t3[:, :, 0:128], in_=d3[:, :, 0:128])
                nc.scalar.dma_start(out=t3[:, :, 128:PAIRS * 128],
                                    in_=d3[:, :, 128:PAIRS * 128])
            else:
                nc.scalar.dma_start(out=t[:], in_=dram[:, :])
            return [t[:, ch * PAIRS * 128:(ch + 1) * PAIRS * 128]
                    for ch in range(CH)]

        wq_sb = load_w(wq, "wq", p0_first=True)
        wk_sb = load_w(wk, "wk", p0_first=True)
        wv_sb = load_w(wv, "wv")

        # xt as ONE SBUF tile, loaded by 3 strided DMAs (3D access pattern
        # covering all 6 chunks per column wave) -- collapses 12 serial
        # ~650ns DMA issues into 3, so the first qkv inputs land ~2us sooner
        xt_t = sb.tile([128, CH * N], dt, name="xt", tag="xt")
        xt3 = xt_t.rearrange("p (c n) -> p c n", n=N)
        xsrc = xt.rearrange("(c p) n -> p c n", p=128)
        nc.sync.dma_start(out=xt3[:, :, 0:512], in_=xsrc[:, :, 0:512])
        nc.sync.dma_start(out=xt3[:, :, 512:1024], in_=xsrc[:, :, 512:1024])
        nc.sync.dma_start(out=xt3[:, :, 1024:N], in_=xsrc[:, :, 1024:N])
        xt_sb = [xt_t[:, ch * N:(ch + 1) * N] for ch in range(CH)]

        wp_t = sb.tile([128, PAIRS * DIM], dt, name="wp", tag="wp")
        nc.scalar.dma_start(out=wp_t[:], in_=wp[:, :])
        wp_sb = [wp_t[:, ch * DIM:(ch + 1) * DIM] for ch in range(PAIRS)]

        # e/v (softmax weights and values) use fp16: exp output is in
        # [0, ~e^5] where fp16 beats bf16 precision, and it matmuls at the
        # same 1 cyc/col.
        edt = mybir.dt.float16 if dt != F32 else F32

        # persistent SBUF tensors
        # v' layout per pair p: cols [p*192 .. p*192+192) = [V_even | 1s | V_odd]
        # (64 cols each).  PV head-a takes cols [0:128) = [V|1s] so its PSUM is
        # [P ; den x64], head-b takes cols [64:192) = [1s|V] giving
        # [den x64 ; P] -- the denominators come out pre-replicated across 64
        # partitions, so normalization needs no gpsimd broadcast / row hop.
        v_sb = [sb.tile([128, PAIRS * 192], edt, name=f"v{k}", tag=f"v{k}")
                for k in range(KT)]
        for k in range(KT):
            v3i = v_sb[k].rearrange("p (g c) -> p g c", c=192)
            nc.vector.memset(v3i[:, :, 64:128], 1.0)
        ot_sb = [sb.tile([128, N], dt, name=f"ot{p}", tag=f"ot{p}")
                 for p in range(PAIRS)]


        # ---- V' (token-major) --------------------------------------------
        def emit_v(kt):
            pv = ps.tile([128, 512], F32, name="dr", tag="dr", bufs=2)
            for ch in range(CH):
                nc.tensor.matmul(
                    pv[:, :PAIRS * 128],
                    lhsT=xt_sb[ch][:, kt * 128:(kt + 1) * 128],
                    rhs=wv_sb[ch][:],
                    start=(ch == 0), stop=(ch == CH - 1),
                )
            v3 = v_sb[kt].rearrange("p (g c) -> p g c", c=192)
            p3 = pv[:, :PAIRS * 128].rearrange("p (g c) -> p g c", c=128)
            nc.vector.tensor_copy(v3[:, :, 0:64], p3[:, :, 0:64])
            nc.vector.tensor_copy(v3[:, :, 128:192], p3[:, :, 64:128])

        qt_tiles = {}
        kt_tiles = {}

        def emit_qkv_group(p, which, qb):
            """One accumulation group: 512 columns of Q^T or K^T for pair p."""
            w_sb, store, nm = (
                (wq_sb, qt_tiles, "qt") if which == 0 else (wk_sb, kt_tiles, "kt")
            )
            if qb == 0:
                store[p] = sb.tile([128, N], dt, name=f"{nm}{p}", tag=nm, bufs=wbufs)
            acc = ps.tile([128, 512], F32, name="dr", tag="dr", bufs=2)
            for ch in range(CH):
                nc.tensor.matmul(
                    acc[:],
                    lhsT=w_sb[ch][:, p * 128:(p + 1) * 128],
                    rhs=xt_sb[ch][:, qb * 512:(qb + 1) * 512],
                    start=(ch == 0), stop=(ch == CH - 1),
                )
            nc.vector.tensor_copy(store[p][:, qb * 512:(qb + 1) * 512], acc[:])

        def emit_proj_group(tt, tail=False):
            # two dr-sized PSUM halves so proj can interleave inside attn
            # blocks without competing for the "s" slots the exp stream
            # needs; the final tranche (no more scores) takes the free "s"
            # slots instead so it pipelines against the last norm chain.
            tsl = slice(tt * 128, (tt + 1) * 128)
            if tail:
                pp = ps.tile([128, 1024], F32, name="s", tag="s", bufs=2)
                pa, pb = pp[:, 0:512], pp[:, 512:768]
            else:
                pa = ps.tile([128, 512], F32, name="dr", tag="dr", bufs=2)
                pb = ps.tile([128, 256], F32, name="dr2", tag="dr", bufs=2)
            for ch in range(PAIRS):
                nc.tensor.matmul(
                    pa[:], lhsT=ot_sb[ch][:, tsl], rhs=wp_sb[ch][:, 0:512],
                    start=(ch == 0), stop=(ch == PAIRS - 1),
                )
                nc.tensor.matmul(
                    pb[:], lhsT=ot_sb[ch][:, tsl], rhs=wp_sb[ch][:, 512:768],
                    start=(ch == 0), stop=(ch == PAIRS - 1),
                )
            st = sb.tile([128, 768], dt, name="st", tag="st", bufs=3)
            if tail:
                # ACT is idle after the last exp: issue the final out stores
                # on the scalar HWDGE ring so they don't serialize behind the
                # sync ring's norm-path DMAs
                nc.vector.tensor_copy(st[:], pp[:, 0:768])
                nc.scalar.dma_start(out=out[tsl, :], in_=st[:])
            else:
                nc.vector.tensor_copy(st[:, 0:512], pa[:])
                nc.vector.tensor_copy(st[:, 512:768], pb[:])
                nc.sync.dma_start(out=out[tsl, :], in_=st[:])

        def attn_begin(p, qb):
            return {
                "p": p, "qb": qb, "e": {},
                "qsl": slice(qb * 512, (qb + 1) * 512),
                "u_a": ps.tile([128, 512], F32, name="ua", tag="u", bufs=2),
                "u_b": ps.tile([128, 512], F32, name="ub", tag="u", bufs=2),
            }

        def attn_scores(ast, kt):
            p, qsl = ast["p"], ast["qsl"]
            qt_t = qt_tiles[p]
            kt_t = kt_tiles[p]
            ksl = slice(kt * 128, (kt + 1) * 128)
            s_ps = ps.tile([128, 1024], F32, name="s", tag="s", bufs=2)
            # scores S^T for both heads, row-tiled (contract=64 each)
            nc.tensor.matmul(
                s_ps[:, 0:512],
                lhsT=kt_t[0:64, ksl], rhs=qt_t[0:64, qsl],
                start=True, stop=True,
            )
            nc.tensor.matmul(
                s_ps[:, 512:1024],
                lhsT=kt_t[64:128, ksl], rhs=qt_t[64:128, qsl],
                start=True, stop=True,
            )
            e_sb = sb.tile([128, 1024], edt, name="e", tag="e", bufs=4)
            nc.scalar.activation(e_sb[:], s_ps[:], EXP, scale=SCALE)
            ast["e"][kt] = e_sb

        def attn_pv(ast, kt):
            p = ast["p"]
            e_sb = ast["e"].pop(kt)
            first = kt == 0
            last = kt == KT - 1
            # PV with the shared ones-block: head-a PSUM = [P1 ; den1 x64],
            # head-b PSUM = [den2 x64 ; P2]
            nc.tensor.matmul(
                ast["u_a"][:, :],
                lhsT=v_sb[kt][:, p * 192:p * 192 + 128],
                rhs=e_sb[:, 0:512],
                start=first, stop=last,
            )
            nc.tensor.matmul(
                ast["u_b"][:, :],
                lhsT=v_sb[kt][:, p * 192 + 64:p * 192 + 192],
                rhs=e_sb[:, 512:1024],
                start=first, stop=last,
            )

        def attn_step(ast, kt):
            attn_scores(ast, kt)
            attn_pv(ast, kt)

        def attn_end_copies(ast, tail=False):
            # one full-tile fp32 copy per U bank (cost is free-size, so
            # [128,512] costs the same as [64,512]) -- frees each PSUM bank
            # with a single DVE op; normalization runs in the NEXT block.
            # The final block skips the copies entirely (no later block needs
            # its U banks) and norms straight out of PSUM, shortening the
            # tail latency chain.
            if tail:
                ast["u_sb"] = None
                return
            u_sb = sb.tile([128, 1024], F32, name="usb", tag="usb", bufs=2)
            nc.vector.tensor_copy(u_sb[:, 0:512], ast["u_a"][:, :])
            nc.vector.tensor_copy(u_sb[:, 512:1024], ast["u_b"][:, :])
            ast["u_sb"] = u_sb

        def attn_norm_recip(ast):
            # dens are pre-replicated across 64 partitions by the ones-block:
            # one full-tile fast reciprocal (custom-DVE ops only work from
            # base partition 0; the non-denominator lanes are never read),
            # then 2 small SBUF->SBUF DMA partition shifts.
            u_sb = ast["u_sb"]
            rr = sb.tile([128, 1024], F32, name="rr", tag="rr", bufs=2)
            rs = sb.tile([128, 512], F32, name="rs", tag="rs", bufs=2)
            if u_sb is None:
                nc.vector.reciprocal_approx_fast(out=rr[:, 0:512],
                                                 in_=ast["u_a"][:, :])
                nc.vector.reciprocal_approx_fast(out=rr[:, 512:1024],
                                                 in_=ast["u_b"][:, :])
            else:
                nc.vector.reciprocal_approx_fast(out=rr[:], in_=u_sb[:])
            nc.sync.dma_start(out=rs[0:64, :], in_=rr[64:128, 0:512])
            nc.sync.dma_start(out=rs[64:128, :], in_=rr[0:64, 512:1024])
            ast["rs"] = rs

        def attn_norm_muls(ast):
            # 2 multiplies writing ot directly for both heads (lane-aligned)
            p, qsl, rs, u_sb = ast["p"], ast["qsl"], ast["rs"], ast["u_sb"]
            ua = ast["u_a"][0:64, :] if u_sb is None else u_sb[0:64, 0:512]
            ub = ast["u_b"][64:128, :] if u_sb is None else u_sb[64:128, 512:1024]
            nc.vector.tensor_mul(ot_sb[p][0:64, qsl], ua, rs[0:64, :])
            nc.vector.tensor_mul(ot_sb[p][64:128, qsl], ub, rs[64:128, :])

        def attn_end_norm(ast):
            attn_norm_recip(ast)
            attn_norm_muls(ast)

        attn_end_norm_tail = attn_end_norm

        def attn_end(ast):
            attn_end_copies(ast)
            attn_end_norm(ast)

        def attn_block(p, qb, fillers=(), prev=None, carry_in=(), tail=False):
            """One attention block with PE filler work woven BETWEEN kt
            steps, so fillers land in the PE's exp-wait slack instead of
            running as a bulk slug that starves the ACT stream. kt steps are
            processed in groups of 2 (4 score MMs, then 4 PV MMs) so the PE
            pays half the tile-mode-switch drains and the score LDWEIGHTS
            pipeline across steps. The previous block's norm chain and
            deferred last filler run early in this block; this block's last
            filler is deferred likewise (PE debt at a block tail delays the
            next block's scores)."""
            fillers = list(fillers)
            keep = max(len(fillers) - 1, 0)
            pos = [((i + 1) * KT) // (keep + 1) for i in range(keep)]
            ast = attn_begin(p, qb)
            fi = 0
            for kt in range(0, KT, 2):
                attn_scores(ast, kt)
                attn_scores(ast, kt + 1)
                attn_pv(ast, kt)
                attn_pv(ast, kt + 1)
                if kt == 0 and prev is not None:
                    attn_end_norm(prev)
                if kt == 0:
                    for f in carry_in:
                        f()
                while fi < keep and pos[fi] <= kt + 1:
                    fillers[fi]()
                    fi += 1
            attn_end_copies(ast, tail=tail)
            while fi < keep:
                fillers[fi]()
                fi += 1
            return ast, fillers[keep:]

        def emit_attn_qb(p, qb):
            ast = attn_begin(p, qb)
            for kt in range(KT):
                attn_step(ast, kt)
            attn_end(ast)

        # ---- schedule ----------------------------------------------------
        if overlap:
            # Slim pipelined head: only Q0(qb0)/K0(qb0)/V'(0:2) precede the
            # first attention steps; remaining K0/Q0/V' weave into the first
            # block so the ACT exp stream starts as early as possible. Later
            # pairs' qkv groups and proj groups weave between the kt steps of
            # subsequent blocks; each block's norm chain runs inside the next.
            #
            # The attention core is SOFTWARE-PIPELINED one group deep: each
            # PV group is emitted after the NEXT scores group, so a new
            # block's first scores precede the previous block's last PVs and
            # the ACT exp stream never stalls at block boundaries.
            emit_qkv_group(0, 0, 0)
            emit_qkv_group(0, 1, 0)
            pend = None          # (ast, kt): PV group not yet emitted
            carry = []
            ast0 = attn_begin(0, 0)
            for k0 in range(0, KT, 2):
                k1 = k0 + 1
                attn_scores(ast0, k0)
                if k1 % 4 == 1 and k1 // 4 < 3:
                    emit_qkv_group(0, 1, k1 // 4 + 1)
                attn_scores(ast0, k1)
                # V' for this group's kt, needed one group later at the PV
                # flush -- emitting after the scores lets the first exp start
                # ~5us earlier
                emit_v(k0)
                emit_v(k1)
                if pend is not None:
                    attn_pv(pend[0], pend[1])
                    attn_pv(pend[0], pend[1] + 1)
                pend = (ast0, k0)
                if k0 == 8:
                    emit_qkv_group(0, 0, 1)

            blocks = []          # (p, qb, fillers, tail)
            # pair-1 prep: only Q/K qb0 must precede block (1,0); K qb1 rides
            # the (0,3)->(1,0) carry, K qb2/qb3 + Q qb1 weave into (1,0) --
            # this drains the PE-oversubscribed pair-0 phase where the ACT
            # exp stream otherwise stalls
            g1 = [(1, w, qb) for w in (0, 1) for qb in range(QB)]
            p0_fill = [[(0, 0, 2), g1[0]], [(0, 0, 3), g1[4]], [g1[1]]]
            for i, qb in enumerate(range(1, QB)):
                blocks.append((0, qb,
                               [lambda g=g: emit_qkv_group(*g) for g in p0_fill[i]],
                               False))
            g2 = [(2, w, qb) for w in (0, 1) for qb in range(QB)]
            p1_fill = [[g1[5], g1[6], g1[7]] + g2[0:2],
                       [g1[2], g1[3]] + g2[2:4], g2[4:6], g2[6:8]]
            for qb in range(QB):
                blocks.append((1, qb,
                               [lambda g=g: emit_qkv_group(*g) for g in p1_fill[qb]],
                               False))
            blocks.append((2, 0, [], False))
            for qb in range(1, QB):
                blocks.append((2, qb, [lambda t=tt: emit_proj_group(t)
                                       for tt in range(4 * (qb - 1), 4 * qb)],
                               qb == QB - 1))

            for (p, qb, fillers, tail) in blocks:
                fillers = list(fillers)
                keep = max(len(fillers) - 1, 0)
                pos = [((i + 1) * KT) // (keep + 1) for i in range(keep)]
                fi = 0
                ast = attn_begin(p, qb)
                for kt in range(0, KT, 2):
                    attn_scores(ast, kt)
                    attn_scores(ast, kt + 1)
                    past, pkt = pend
                    attn_pv(past, pkt)
                    attn_pv(past, pkt + 1)
                    pend = (ast, kt)
                    if pkt == KT - 2:
                        # previous block fully accumulated: evacuate + norm,
                        # then its deferred carry filler
                        attn_end_copies(past)
                        attn_end_norm(past)
                        for f in carry:
                            f()
                        carry = []
                    while fi < keep and pos[fi] <= kt + 1:
                        fillers[fi]()
                        fi += 1
                carry = carry + fillers[keep:]
            # drain the pipeline: last block's final PV group.  kt=15 is
            # split so each U bank's reciprocal (straight out of PSUM)
            # starts while the other head's PV still streams, shortening
            # the tail norm latency chain.
            past, pkt = pend
            attn_pv(past, pkt)
            e_l = past["e"].pop(pkt + 1)
            p_l, qsl_l = past["p"], past["qsl"]
            nc.tensor.matmul(
                past["u_a"][:, :],
                lhsT=v_sb[pkt + 1][:, p_l * 192:p_l * 192 + 128],
                rhs=e_l[:, 0:512], start=False, stop=True,
            )
            rr_l = sb.tile([128, 1024], F32, name="rr", tag="rr", bufs=2)
            nc.vector.reciprocal_approx_fast(out=rr_l[:, 0:512],
                                             in_=past["u_a"][:, :])
            nc.tensor.matmul(
                past["u_b"][:, :],
                lhsT=v_sb[pkt + 1][:, p_l * 192 + 64:p_l * 192 + 192],
                rhs=e_l[:, 512:1024], start=False, stop=True,
            )
            nc.vector.reciprocal_approx_fast(out=rr_l[:, 512:1024],
                                             in_=past["u_b"][:, :])
            rs_l = sb.tile([128, 512], F32, name="rs", tag="rs", bufs=2)
            nc.sync.dma_start(out=rs_l[0:64, :], in_=rr_l[64:128, 0:512])
            nc.sync.dma_start(out=rs_l[64:128, :], in_=rr_l[0:64, 512:1024])
            for f in carry:
                f()
            nc.vector.tensor_mul(ot_sb[p_l][0:64, qsl_l],
                                 past["u_a"][0:64, :], rs_l[0:64, :])
            nc.vector.tensor_mul(ot_sb[p_l][64:128, qsl_l],
                                 past["u_b"][64:128, :], rs_l[64:128, :])
            # alternate the final proj groups between the free "s" slots and
            # the "dr" halves so they pipeline on independent PSUM banks
            for tt in range(4 * (QB - 1), 4 * QB):
                emit_proj_group(tt, tail=(tt % 2 == 0))
        else:
            for kt in range(KT):
                emit_v(kt)
            for p in range(PAIRS):
                for which in (0, 1):
                    for qb in range(QB):
                        emit_qkv_group(p, which, qb)
                for qb in range(QB):
                    emit_attn_qb(p, qb)
            for tt in range(KT):
                emit_proj_group(tt)


_NC = {}


def _get_nc(dtype, overlap=None):
    key = (dtype, overlap)
    if key not in _NC:
        _NC[key] = build_program(dtype, overlap=overlap)
    return _NC[key]


def make_in_maps(x, w_qkv, w_proj, dtype):
    np_dt = np.float32 if dtype == "f32" else ml_dtypes.bfloat16
    def merge(w):
        # [CH*128, K] -> [128, CH*K]: row r gets all chunks side by side
        chn = w.shape[0] // 128
        return np.ascontiguousarray(
            w.reshape(chn, 128, w.shape[1]).transpose(1, 0, 2)
            .reshape(128, chn * w.shape[1]))

    in_maps = []
    for c in range(NCORES):
        b = c // 2
        h0 = (c % 2) * 6 * HD
        in_maps.append({
            "xt": np.ascontiguousarray(x[b].T).astype(np_dt),
            "wq": merge(w_qkv[:, h0:h0 + 384]).astype(np_dt),
            "wk": merge(w_qkv[:, DIM + h0:DIM + h0 + 384]).astype(np_dt),
            "wv": merge(w_qkv[:, 2 * DIM + h0:2 * DIM + h0 + 384]).astype(np_dt),
            "wp": merge(w_proj[h0:h0 + 384, :]).astype(np_dt),
        })
    return in_maps


def run(x, w_qkv, w_proj, b_proj, trace=False, dtype=None, overlap=None):
    dtype = dtype or DEFAULT_DTYPE
    x = np.asarray(x, dtype=np.float32)
    w_qkv = np.asarray(w_qkv, dtype=np.float32)
    w_proj = np.asarray(w_proj, dtype=np.float32)
    b_proj = np.asarray(b_proj, dtype=np.float32)

    in_maps = make_in_maps(x, w_qkv, w_proj, dtype)
    res = run_bass_kernel_spmd(_get_nc(dtype, overlap), in_maps, list(range(NCORES)),
                               trace=trace)
    full = np.empty((B, N, DIM), dtype=np.float32)
    for b in range(B):
        full[b] = (res.results[2 * b]["out"].astype(np.float32)
                   + res.results[2 * b + 1]["out"].astype(np.float32) + b_proj)
    return full, res


def kernel(x, w_qkv, w_proj, b_proj):
    full, _ = run(x, w_qkv, w_proj, b_proj, trace=False)
    return full

